# revision 1
# baseline (speedup 1.0000x reference)
"""CavityLoss Trainium2 kernel (nn_CavityLoss_43722767073667), v6.

Mathematical reduction of the reference, exact in fp32 (verified against a
bit-faithful numpy emulation incl. adversarial threshold-boundary values):

  pb = (floor(pred*255) >= 128)  <=>  (pred >= c*),  c* = f32(128/255)
  The 5^3 all-ones dilation of the binary gt is an exact integer count
  >= gt (the window contains the center voxel), so
      diff = ((gt - pb*dilate(gt)) > 0) == gt * (1 - pb)     [identity]
  Non-critical voxels contribute exactly 0 to the BCE in fp32, so
      loss = -mean( gt * [pred < c*] * ln(pred) ).

v6 "gate-before-Ln" (vs the two-STT baseline): both masks apply BEFORE the
log, so the ACT Ln's own accumulator produces the per-tile row sums and the
post-Ln DVE pass disappears from the critical chain:

  STT#1 (DVE): r  = (p is_ge c*) max p      # p if p < c*, else exactly 1.0
  STT#2 (DVE): r2 = (gt is_lt 0.5) max r    # r if gt == 1, else exactly 1.0
                                            #   (r <= 1.0 always, so max(1,r)=1)
  ACT:  acc[:,t] = sum Ln(r2)               # ln(p) on critical voxels;
                                            # Ln_table(1.0) elsewhere
  Ln_table(1.0) <= 4e-7 is PROVEN by the baseline's own measurement: it
  summed Ln_table(1.0) * gt over ~1.06M masked voxels (via its post-Ln
  gt-multiply) and still measured rel err 2.4e-7. Residual here:
  eps * 6.0M / 1.79M <= ~1.3e-6 relative, far under the 2e-2 gate.

  ones^T @ acc on the idle PE reduces partitions -> [1, NT], one 20-byte
  contiguous DMA out; host combines in f64: loss = -sum(acc)/N.

Distribution: 192^3 volume flattened and split into 8 equal slabs (depth
sharding: 24 z-planes per core), each viewed as [128 partitions, 6912].
Pointwise + reduction only - the dilation cancels, so no halo exchange and
no collectives.

Scheduling notes (v6, from HW trace analysis):
  - DMA completion semaphores release ~1.4us (early stream) to ~3-5us (late
    stream) AFTER the last data byte. Transfer order on the single sync
    HWDGE ring is [p0, p1, g0, p2, g1, p3, g2, p4, g3, g4]: each pred tile
    arrives one slot earlier than the baseline's strict pairing, so the
    pred-keyed STT#1 chain front-loads while gt tiles (whose consumer STT#2
    is cheap) absorb the late-stream lag.
  - per-tile chain is STT#1 -> STT#2 -> Ln(accum): ACT is TERMINAL, so the
    tail is one engine hop shorter than the baseline's STT#1->Ln->STT#2.
  - DVE emission order interleaves STT#2(t) between STT#1(t+1)/STT#1(t+2)
    to match the shifted arrival schedule.
  - dummy Ln on the const-1.0 tile hoists the ~1.3us ACT_TABLE_LOAD into
    the DMA window; progressive tile sizes keep the final chain short.
"""

import numpy as np

import concourse.bacc as bacc
import concourse.mybir as mybir
from concourse.bass_utils import run_bass_kernel_spmd

D = 192
N_CORES = 8
P = 128
TOTAL = D * D * D              # 7_077_888
PER_CORE = TOTAL // N_CORES    # 884_736
FREE = PER_CORE // P           # 6_912
SIZES = [1728, 1728, 1728, 1152, 576]
assert sum(SIZES) == FREE
NT = len(SIZES)

C_STAR = float(np.float32(128.0) / np.float32(255.0))

_CACHE = {}


def _build():
    nc = bacc.Bacc("TRN2", name="cavity_loss")
    f32 = mybir.dt.float32
    pred = nc.dram_tensor("pred", [P, FREE], f32, kind="ExternalInput")
    gt = nc.dram_tensor("gt", [P, FREE], f32, kind="ExternalInput")
    out = nc.dram_tensor("out", [1, NT], f32, kind="ExternalOutput")

    ge = mybir.AluOpType.is_ge
    lt = mybir.AluOpType.is_lt
    mx = mybir.AluOpType.max
    Ln = mybir.ActivationFunctionType.Ln

    pred_sb = nc.alloc_sbuf_tensor("pred_sb", [P, FREE], f32).ap()
    gt_sb = nc.alloc_sbuf_tensor("gt_sb", [P, FREE], f32).ap()
    r_sb = nc.alloc_sbuf_tensor("r_sb", [P, FREE], f32).ap()
    r2_sb = nc.alloc_sbuf_tensor("r2_sb", [P, FREE], f32).ap()
    ln_sb = nc.alloc_sbuf_tensor("ln_sb", [P, max(SIZES)], f32).ap()  # scratch
    acc = nc.alloc_sbuf_tensor("acc_sb", [P, NT], f32).ap()

    s_pred = [nc.alloc_semaphore(f"s_pred{t}") for t in range(NT)]
    s_gt = [nc.alloc_semaphore(f"s_gt{t}") for t in range(NT)]
    s_r2 = nc.alloc_semaphore("s_r2")
    s_fin = nc.alloc_semaphore("s_fin")
    s_mm = nc.alloc_semaphore("s_mm")
    s_cp = nc.alloc_semaphore("s_cp")
    s_out = nc.alloc_semaphore("s_out")

    offs = np.concatenate([[0], np.cumsum(SIZES)]).tolist()
    sls = [slice(offs[t], offs[t + 1]) for t in range(NT)]

    # single sync HWDGE ring; pred tiles pulled one slot earlier than the
    # strict p,g pairing so the pred-keyed STT#1 chain front-loads
    def dma_p(t):
        nc.sync.dma_start(pred_sb[:, sls[t]], pred[:, sls[t]]).then_inc(
            s_pred[t], 16
        )

    def dma_g(t):
        nc.sync.dma_start(gt_sb[:, sls[t]], gt[:, sls[t]]).then_inc(s_gt[t], 16)

    dma_p(0)
    dma_p(1)
    for t in range(2, NT):
        dma_g(t - 2)
        dma_p(t)
    dma_g(NT - 2)
    dma_g(NT - 1)

    # scalar: dummy Ln pulls ACT_TABLE_LOAD into the DMA window, then the
    # per-tile Ln+accum chain (single wait: r2 implies pred+gt arrived)
    dummy = nc.alloc_sbuf_tensor("dummy_sb", [P, 1], f32).ap()
    nc.scalar.activation(dummy[:], nc.const_aps.tensor(1.0, (P, 1)), Ln)
    for t in range(NT):
        sl = sls[t]
        W = SIZES[t]
        nc.scalar.wait_ge(s_r2, t + 1)
        a = nc.scalar.activation(
            ln_sb[:, :W], r2_sb[:, sl], Ln, accum_out=acc[:, t : t + 1]
        )
    a.then_inc(s_fin, 1)

    # vector: STT#1 (pred-keyed) and STT#2 (gt-keyed), software-pipelined to
    # match the shifted transfer order
    def stt1(t):
        sl = sls[t]
        nc.vector.wait_ge(s_pred[t], 16)
        nc.vector.scalar_tensor_tensor(
            r_sb[:, sl], pred_sb[:, sl], C_STAR, pred_sb[:, sl], ge, mx
        )

    def stt2(t):
        sl = sls[t]
        nc.vector.wait_ge(s_gt[t], 16)
        nc.vector.scalar_tensor_tensor(
            r2_sb[:, sl], gt_sb[:, sl], 0.5, r_sb[:, sl], lt, mx
        ).then_inc(s_r2, 1)

    stt1(0)
    stt1(1)
    stt2(0)
    for t in range(2, NT):
        stt1(t)
        stt2(t - 1)
    stt2(NT - 1)

    # finalize: partition-reduce acc on the (otherwise idle) TensorEngine,
    # then one contiguous tiny DMA: [1, NT] on one partition = 1 descriptor
    psum_fin = nc.alloc_psum_tensor("psum_fin", [1, NT], f32).ap()
    fin_sb = nc.alloc_sbuf_tensor("fin_sb", [1, NT], f32).ap()
    ones = nc.const_aps.tensor(1.0, (P, 1))
    nc.tensor.wait_ge(s_fin, 1)
    nc.tensor.matmul(psum_fin[:], ones, acc[:], start=True, stop=True).then_inc(
        s_mm, 1
    )
    nc.vector.wait_ge(s_mm, 1)
    nc.vector.tensor_copy(fin_sb[:], psum_fin[:]).then_inc(s_cp, 1)
    nc.sync.wait_ge(s_cp, 1)
    nc.sync.dma_start(out[:], fin_sb[:]).then_inc(s_out, 16)
    nc.sync.wait_ge(s_out, 16)

    nc.compile()
    return nc


def _get_nc():
    if "nc" not in _CACHE:
        _CACHE["nc"] = _build()
    return _CACHE["nc"]


def _shard(x):
    flat = np.ascontiguousarray(np.asarray(x, dtype=np.float32)).reshape(-1)
    assert flat.size == TOTAL, f"expected {TOTAL} elements, got {flat.size}"
    return [
        flat[c * PER_CORE : (c + 1) * PER_CORE].reshape(P, FREE)
        for c in range(N_CORES)
    ]


def run_spmd(pred, gt, **kw):
    """Shard, run on 8 cores; returns BassKernelResults (kw e.g. trace=True)."""
    preds = _shard(pred)
    gts = _shard(gt)
    in_maps = [{"pred": preds[c], "gt": gts[c]} for c in range(N_CORES)]
    return run_bass_kernel_spmd(
        _get_nc(), in_maps, core_ids=list(range(N_CORES)), **kw
    )


def kernel(pred, gt):
    res = run_spmd(pred, gt)
    total = 0.0
    for r in res.results:
        total += float(r["out"].astype(np.float64).sum())
    return np.asarray(np.float32(-total / TOTAL))

